# revision 34
# baseline (speedup 1.0000x reference)
"""MultiHeadGAT Trainium2 kernel (8 NeuronCores, data-parallel over batch).

Reference computation (per batch b of 32, n=512 nodes, d=128 feats, H=8 heads,
HID=64, top-k=16, leaky=0.2):
    h' = (h @ W).reshape(n, H, HID)                      # projection
    ei[g,i] = h'[i,g,:] . a_i[g];  ej[g,j] = h'[j,g,:] . a_j[g]
    e[g,i,j] = leaky_relu(ei[g,i] + ej[g,j])
    mask = topk_16(e, axis=j) | eye(n)
    attn = softmax(where(mask, e, -1e9))
    out = elu(attn @ h')

Structure exploited (rank-17 trick): the top-16 column set J_g is
row-independent (leaky_relu is monotone), so attention reduces to 16 shared
candidates per head plus the diagonal.

Candidate scores live in a TRANSPOSED layout [(head,cand)=128 part x n=512]:
  * cand = t_c + ei_n comes out of ONE K=64 PE matmul (stacked selector)
  * exp(leaky(cand)) = max(exp(s), exp(0.2 s)) - two ACTs + one bf16 max
  * the result IS the stationary operand of the output matmul (no transposes)
Other key moves:
  * V gather via associativity: V = (h^T S)^T @ W  (gather in d=128 space)
  * diagonal term dt = pd * h' reads the projection straight out of PSUM
    (projection runs late; no separate PSUM->SBUF evict pass)
  * softmax reciprocal via DVE reciprocal_approx_fast (no ACT table switch)
  * den for all 4 batches accumulated into one [32,N] PSUM group via
    per-batch block-selector lhsT constants
  * ELU via elu(y) = min(exp(y)-1, relu(y)); per-batch variant split
    balances Scalar vs Vector load
  * program order keeps PE ahead of Scalar/DVE consumers (HAM warm)
"""
import sys
import numpy as np

sys.path.insert(0, "/opt/trn_rl_repo")

from contextlib import ExitStack

import concourse.bass as bass
import concourse.tile as tile
from concourse import bacc, mybir
from concourse.bass_utils import run_bass_kernel_spmd

f32 = mybir.dt.float32
bf16 = mybir.dt.bfloat16
AX = mybir.AxisListType
ALU = mybir.AluOpType
AF = mybir.ActivationFunctionType

N_HEADS = 8
HID = 64
TOP_K = 16
SLOPE = 0.2
BS, N, D = 32, 512, 128
CORES = 8
BPC = BS // CORES          # batches per core = 4
NCH = N // 128             # n-chunks = 4
GD = N_HEADS * HID         # 512


def _bc(ap, insert_at, counts_steps):
    """Insert [step, count] dims into an AP at position insert_at."""
    new = list(ap.ap)
    for step, count in reversed(counts_steps):
        new.insert(insert_at, [step, count])
    return bass.AP(ap.tensor, ap.offset, new)


def build_graph():
    nc = bacc.Bacc("TRN2", target_bir_lowering=False, debug=False)

    hT_ext = nc.dram_tensor("hT", [BPC, D, N], f32, kind="ExternalInput")
    hTb_ext = nc.dram_tensor("hTb", [BPC, D, N], bf16, kind="ExternalInput")
    hnat_ext = nc.dram_tensor("hnat", [BPC, 128, NCH, D], bf16,
                              kind="ExternalInput")
    W_ext = nc.dram_tensor("W", [D, GD], bf16, kind="ExternalInput")
    P_ext = nc.dram_tensor("P", [D, 24], f32, kind="ExternalInput")
    # host-precomputed constants
    rowsel_ext = nc.dram_tensor("rowsel", [32, BPC * 128], bf16,
                                kind="ExternalInput")
    ident_ext = nc.dram_tensor("ident", [128, 128], f32, kind="ExternalInput")
    identb_ext = nc.dram_tensor("identb", [128, 128], bf16,
                                kind="ExternalInput")
    mblk_ext = nc.dram_tensor("mblk", [128, GD], bf16, kind="ExternalInput")
    b2_ext = nc.dram_tensor("B2", [128, BPC, 32], bf16, kind="ExternalInput")
    bsel_ext = nc.dram_tensor("Bsel", [32, BPC * 128], bf16,
                              kind="ExternalInput")
    ones_ext = nc.dram_tensor("ones32", [32, N], bf16, kind="ExternalInput")
    onesf_ext = nc.dram_tensor("onesf", [32, 128], f32, kind="ExternalInput")
    out_ext = nc.dram_tensor("out", [BPC, N, N_HEADS, HID], bf16,
                             kind="ExternalOutput")
    hT = hT_ext.ap()
    hTb = hTb_ext.ap()
    hnat = hnat_ext.ap()
    outap = out_ext.ap()

    with tile.TileContext(nc) as tc, ExitStack() as ctx:
        const = ctx.enter_context(tc.tile_pool(name="const", bufs=1))
        sb = ctx.enter_context(tc.tile_pool(name="sb", bufs=2))
        sb3 = ctx.enter_context(tc.tile_pool(name="sb3", bufs=3))
        sb4 = ctx.enter_context(tc.tile_pool(name="sb4", bufs=BPC))
        # PSUM (8 banks): o 2x2 + mm 2x1 + sm 1 + den 1
        ps = ctx.enter_context(tc.tile_pool(name="ps", bufs=2, space="PSUM"))

        # ---------------- constants (host-precomputed, DMA'd) ----------
        W_sb = const.tile([128, GD], bf16)
        nc.sync.dma_start(W_sb[:], W_ext.ap())
        P_sb = const.tile([128, 24], f32)
        nc.sync.dma_start(P_sb[:], P_ext.ap())
        ident = const.tile([128, 128], f32)
        nc.sync.dma_start(ident[:], ident_ext.ap())
        identb = const.tile([128, 128], bf16)
        nc.sync.dma_start(identb[:], identb_ext.ap())
        mblk = const.tile([128, GD], bf16)
        nc.sync.dma_start(mblk[:], mblk_ext.ap())
        B2 = const.tile([128, BPC, 32], bf16)
        nc.sync.dma_start(B2[:], b2_ext.ap())
        Bsel = const.tile([32, BPC * 128], bf16)
        nc.sync.dma_start(Bsel[:], bsel_ext.ap())
        # s-matmul operands in stacked hi+lo bf16 (f32 PE matmuls run as
        # 2-pass LOW_HIGH at ~8x the bf16 cost).
        # EIones_*: rows 0:32 = ones (pairs rhsb_*), rows 32:64 = EI_hi/lo
        EIones_hi = const.tile([64, N], bf16)
        nc.sync.dma_start(EIones_hi[0:32, :], ones_ext.ap())
        EIones_lo = const.tile([64, N], bf16)
        nc.sync.dma_start(EIones_lo[0:32, :], ones_ext.ap())
        onesf = const.tile([32, 128], f32)
        nc.sync.dma_start(onesf[:], onesf_ext.ap())
        # srhs_*: rows 0:32 = rhsb_hi/lo (DVE), rows 32:64 = rowsel (const)
        srhs_hi, srhs_lo = [], []
        for b in range(BPC):
            srh = sb4.tile([64, 128], bf16, tag="srhsh", name=f"srhsh{b}")
            nc.sync.dma_start(srh[32:64, :],
                              rowsel_ext.ap()[:, b * 128:(b + 1) * 128])
            srhs_hi.append(srh)
            srl = sb4.tile([64, 128], bf16, tag="srhsl", name=f"srhsl{b}")
            nc.sync.dma_start(srl[32:64, :],
                              rowsel_ext.ap()[:, b * 128:(b + 1) * 128])
            srhs_lo.append(srl)

        # ---------------- stage A: scores ------------------------------
        T = const.tile([32, N], f32)       # ej rows: (b,g) x n
        sd = const.tile([32, N], f32)      # ei+ej rows (diag scores)
        EI = const.tile([32, N], f32)      # ei rows
        hT_sb = []
        hTb_sb = []
        hnat_sb = []
        sc_sb = []
        for b in range(BPC):
            ht = sb4.tile([128, N], f32, tag="ht")
            nc.sync.dma_start(ht[:], hT[b])
            htb = sb4.tile([128, N], bf16, tag="htb")
            nc.sync.dma_start(htb[:], hTb[b])
            hnt = sb4.tile([128, NCH, D], bf16, tag="hnat")
            nc.sync.dma_start(hnt[:], hnat[b])
            hT_sb.append(ht)
            hTb_sb.append(htb)
            hnat_sb.append(hnt)

            # scores EIJT[(ij,g), n] = P^T @ hT   (f32; bf16 scores fail:
            # 0.16 rel err from top-k set instability)
            sc_ps = ps.tile([24, N], f32, tag="mm")
            nc.tensor.matmul(sc_ps[:], P_sb[:], ht[:], start=True, stop=True)
            sc = sb4.tile([24, N], f32, tag="sc")
            nc.vector.tensor_copy(sc[:], sc_ps[:])
            sc_sb.append(sc)
            nc.gpsimd.dma_start(T[b * 8:(b + 1) * 8, :], sc[0:8, :])
            nc.gpsimd.dma_start(EI[b * 8:(b + 1) * 8, :], sc[8:16, :])
            nc.gpsimd.dma_start(sd[b * 8:(b + 1) * 8, :], sc[16:24, :])

        # ---------------- stage B: top-16 + diagonal terms -------------
        vals = const.tile([32, 16], f32)
        T2 = const.tile([32, N], f32)
        nc.vector.max(vals[:, 0:8], T[:])
        nc.vector.match_replace(T2[:], vals[:, 0:8], T[:], -1e30)
        nc.vector.max(vals[:, 8:16], T2[:])

        # diag: pdiag = exp(leaky(ei+ej)) * (ej < t16)   [32, 512] folded
        sdl = const.tile([32, N], f32)
        nc.vector.scalar_tensor_tensor(sdl[:], sd[:], SLOPE, sd[:],
                                       op0=ALU.mult, op1=ALU.max)
        expd = const.tile([32, N], f32)
        nc.scalar.activation(expd[:], sdl[:], AF.Exp)
        pdiag = const.tile([32, N], f32)
        nc.vector.scalar_tensor_tensor(pdiag[:], T[:], vals[:, 15:16],
                                       expd[:], op0=ALU.is_lt, op1=ALU.mult)

        # hi/lo splits for the bf16 s-matmul and rbc-matmul
        EIh = const.tile([32, N], bf16)
        nc.vector.tensor_copy(EIh[:], EI[:])
        EIl = const.tile([32, N], bf16)
        nc.vector.tensor_tensor(EIl[:], EI[:], EIh[:], op=ALU.subtract)
        nc.gpsimd.dma_start(EIones_hi[32:64, :], EIh[:])
        nc.gpsimd.dma_start(EIones_lo[32:64, :], EIl[:])
        vals_h = const.tile([32, 16], bf16)
        nc.vector.tensor_copy(vals_h[:], vals[:])
        vals_l = const.tile([32, 16], bf16)
        nc.vector.tensor_tensor(vals_l[:], vals[:], vals_h[:],
                                op=ALU.subtract)

        # ej in node layout for the one-hot gather: 4 transposes of T
        ejt_ps = ps.tile([128, NCH, 32], f32, tag="sm", bufs=1)
        for c in range(NCH):
            nc.tensor.transpose(ejt_ps[:, c, :],
                                T[:, c * 128:(c + 1) * 128],
                                ident[0:32, 0:32])
        ejn = const.tile([128, NCH, 32], f32)
        nc.vector.tensor_copy(ejn[:], ejt_ps[:])

        # ------------- stage C1a per batch: scores -> pex, S -----------
        pex_sb = []
        S_sb = []
        for b in range(BPC):
            # rhsb_*[k,(g,c)] = vals_*[k,c] * (k == b*8+g)
            id_sl = ident[0:32, b * 8:(b + 1) * 8].broadcast_to([32, 8, 16])
            nc.vector.tensor_tensor(
                srhs_hi[b][0:32, :].rearrange("k (g c) -> k g c", g=N_HEADS),
                _bc(vals_h[:, 0:16], 1, [[0, N_HEADS]]), id_sl, op=ALU.mult)
            nc.vector.tensor_tensor(
                srhs_lo[b][0:32, :].rearrange("k (g c) -> k g c", g=N_HEADS),
                _bc(vals_l[:, 0:16], 1, [[0, N_HEADS]]), id_sl, op=ALU.mult)

            # s[(g,c), n] = ei[b,g,n] + t[(g,c)] as hi+lo bf16 pair
            s_ps = ps.tile([128, N], f32, tag="mm")
            nc.tensor.matmul(s_ps[:], srhs_hi[b][:], EIones_hi[:],
                             start=True, stop=False)
            nc.tensor.matmul(s_ps[:], srhs_lo[b][:], EIones_lo[:],
                             start=False, stop=True)

            # one-hot prep: vbc[p,(g,c)] = vals[b*8+g, c] broadcast
            # (must be bit-exact vals for the f32 is_equal match -> f32 mm)
            rhsbf = sb.tile([32, N_HEADS, 16], f32, tag="rhsbf")
            nc.vector.tensor_tensor(
                rhsbf[:], _bc(vals[:, 0:16], 1, [[0, N_HEADS]]), id_sl,
                op=ALU.mult)
            vbc_ps = ps.tile([128, 128], f32, tag="sm", bufs=1)
            nc.tensor.matmul(vbc_ps[:], onesf[:],
                             rhsbf[:].rearrange("k g c -> k (g c)"),
                             start=True, stop=True)
            vbc = sb.tile([128, 128], f32, tag="vbc")
            nc.vector.tensor_copy(vbc[:], vbc_ps[:])

            # exp(leaky(s)) = max(exp(s), exp(0.2 s))
            e1 = sb.tile([128, N], bf16, tag="e1")
            nc.scalar.activation(e1[:], s_ps[:], AF.Exp)
            e2 = sb.tile([128, N], bf16, tag="e2")
            nc.scalar.activation(e2[:], s_ps[:], AF.Exp, scale=SLOPE)
            pex = sb4.tile([128, N], bf16, tag="pex")
            nc.vector.tensor_tensor(pex[:], e1[:], e2[:], op=ALU.max)
            pex_sb.append(pex)

            # one-hot S[n, (g,c)] = (ej[n,g] == vals[b*8+g, c])
            S = sb4.tile([128, NCH, 128], bf16, tag="S")
            ej_bc = _bc(ejn[:, :, b * 8:(b + 1) * 8], 3, [[0, 16]])
            vbc_bc = _bc(vbc[:].rearrange("p (g c) -> p g c", g=N_HEADS),
                         1, [[0, NCH]])
            nc.vector.tensor_tensor(
                S[:].rearrange("p c (g x) -> p c g x", g=N_HEADS),
                ej_bc, vbc_bc, op=ALU.is_equal)
            S_sb.append(S)

        # ------------- stage C1b per batch: V gather + den -------------
        den_ps = ps.tile([32, N], f32, tag="den", bufs=1)
        hblk_sb = []
        for b in range(BPC):
            # V = (h^T S)^T W ; gather in d-space then project
            hS_ps = ps.tile([128, 128], f32, tag="sm", bufs=1)
            for c in range(NCH):
                nc.tensor.matmul(hS_ps[:], hnat_sb[b][:, c, :],
                                 S_sb[b][:, c, :],
                                 start=(c == 0), stop=(c == NCH - 1))
            hS = sb.tile([128, 128], bf16, tag="hS")
            nc.vector.tensor_copy(hS[:], hS_ps[:])
            V_ps = ps.tile([128, GD], f32, tag="mm")
            nc.tensor.matmul(V_ps[:], hS[:], W_sb[:], start=True, stop=True)
            hblk = sb4.tile([128, GD], bf16, tag="hblk")
            nc.vector.tensor_tensor(hblk[:], V_ps[:], mblk[:], op=ALU.mult)
            hblk_sb.append(hblk)

            # den[(b,g), n] = sum_c pex, accumulated into one [32,N] group
            nc.tensor.matmul(den_ps[:], B2[:, b, :], pex_sb[b][:],
                             start=(b == 0), stop=(b == BPC - 1),
                             skip_group_check=True)

        # ---------------- stage R: reciprocal + pd (global) ------------
        denf = const.tile([32, N], f32)
        nc.vector.tensor_tensor(denf[:], den_ps[:], pdiag[:], op=ALU.add)
        recip = const.tile([32, N], f32)
        nc.vector.reciprocal_approx_fast(recip[:], denf[:])
        r_hi = const.tile([32, N], bf16)
        nc.vector.tensor_copy(r_hi[:], recip[:])
        r_lo = const.tile([32, N], bf16)
        nc.vector.tensor_tensor(r_lo[:], recip[:], r_hi[:], op=ALU.subtract)
        pd_t = const.tile([32, N], bf16)
        nc.gpsimd.tensor_tensor(pd_t[:], pdiag[:], recip[:], op=ALU.mult)

        # pdn[n, (c, b, g)]: 4 PE transposes of pd_t
        pdn_ps = ps.tile([128, NCH, 32], bf16, tag="sm", bufs=1)
        for c in range(NCH):
            nc.tensor.transpose(pdn_ps[:, c, :],
                                pd_t[:, c * 128:(c + 1) * 128],
                                identb[0:32, 0:32])
        pdn = const.tile([128, NCH, 32], bf16)
        nc.vector.tensor_copy(pdn[:], pdn_ps[:])

        # ---------------- stage C2 per batch: qn, proj, out, elu -------
        qn_sb = []
        for b in range(BPC):
            # recip broadcast to (g,c) partitions; qn = pex * recip
            rbc_ps = ps.tile([128, N], f32, tag="mm")
            nc.tensor.matmul(rbc_ps[:], Bsel[:, b * 128:(b + 1) * 128],
                             r_hi[:], start=True, stop=False)
            nc.tensor.matmul(rbc_ps[:], Bsel[:, b * 128:(b + 1) * 128],
                             r_lo[:], start=False, stop=True)
            qn = sb4.tile([128, N], bf16, tag="qn")
            nc.vector.tensor_tensor(qn[:], pex_sb[b][:], rbc_ps[:],
                                    op=ALU.mult)
            qn_sb.append(qn)

        for b in range(BPC):
            qn = qn_sb[b]
            # (re)projection per half; dt = pd * h' straight from PSUM
            dt = sb.tile([128, NCH, GD], bf16, tag="dt")
            for h in range(2):
                hp_ps = ps.tile([128, 2, GD], f32, tag="o")
                for k in range(2):
                    c = 2 * h + k
                    nc.tensor.matmul(hp_ps[:, k, :],
                                     hTb_sb[b][:, c * 128:(c + 1) * 128],
                                     W_sb[:], start=True, stop=True)
                pdn_b = _bc(
                    _bc(pdn[:, 2 * h:2 * h + 2, b * 8:(b + 1) * 8],
                        3, [[0, HID]]), 0, [])
                nc.vector.tensor_tensor(
                    dt[:, 2 * h:2 * h + 2, :].rearrange(
                        "p c (g x) -> p c g x", g=N_HEADS),
                    hp_ps[:].rearrange("p c (g x) -> p c g x", g=N_HEADS),
                    pdn_b, op=ALU.mult)

            # output halves: attn matmul + diag add, then ELU
            for h in range(2):
                o_ps = ps.tile([128, 2, GD], f32, tag="o")
                for k in range(2):
                    c = 2 * h + k
                    nc.tensor.matmul(o_ps[:, k, :],
                                     qn[:, c * 128:(c + 1) * 128],
                                     hblk_sb[b][:], start=True, stop=False)
                for k in range(2):
                    c = 2 * h + k
                    nc.tensor.matmul(o_ps[:, k, :], identb[:], dt[:, c, :],
                                     start=False, stop=True)
                vex = sb3.tile([128, 2, GD], bf16, tag="vex")
                nc.scalar.activation(vex[:], o_ps[:], AF.Exp)
                ot = sb3.tile([128, 2, GD], bf16, tag="ot")
                if b < 2:
                    # scalar-heavy: ry on ACT, all-bf16 STT
                    ry = sb3.tile([128, 2, GD], bf16, tag="ry")
                    nc.scalar.activation(ry[:], o_ps[:], AF.Relu)
                    nc.vector.scalar_tensor_tensor(ot[:], vex[:], 1.0, ry[:],
                                                   op0=ALU.subtract,
                                                   op1=ALU.min)
                else:
                    # vector-heavy: vexm1 via 4x TS, mixed STT vs PSUM
                    vm1 = sb3.tile([128, 2, GD], bf16, tag="vm1")
                    nc.vector.tensor_scalar(vm1[:], vex[:], 1.0, None,
                                            op0=ALU.subtract)
                    nc.vector.scalar_tensor_tensor(ot[:], o_ps[:], 0.0,
                                                   vm1[:], op0=ALU.max,
                                                   op1=ALU.min)
                nc.sync.dma_start(
                    outap[b, 2 * h * 128:(2 * h + 2) * 128].rearrange(
                        "(k p) g x -> p k g x", k=2),
                    ot[:].rearrange("p k (g x) -> p k g x", g=N_HEADS))

    nc.compile()
    return nc


_CACHE = {}


def _get_graph():
    if "nc" not in _CACHE:
        _CACHE["nc"] = build_graph()
    return _CACHE["nc"]


def _consts():
    import ml_dtypes
    rowsel = np.zeros((32, BPC * 128), np.float32)
    for b in range(BPC):
        for g in range(N_HEADS):
            rowsel[b * 8 + g, b * 128 + g * 16:b * 128 + (g + 1) * 16] = 1.0
    ident = np.eye(128, dtype=np.float32)
    mblk = np.zeros((128, GD), np.float32)
    for g in range(N_HEADS):
        mblk[g * 16:(g + 1) * 16, g * HID:(g + 1) * HID] = 1.0
    B2 = np.zeros((128, BPC, 32), np.float32)
    for b in range(BPC):
        for g in range(N_HEADS):
            B2[g * 16:(g + 1) * 16, b, b * 8 + g] = 1.0
    Bsel = np.zeros((32, BPC * 128), np.float32)
    for b in range(BPC):
        for g in range(N_HEADS):
            Bsel[b * 8 + g, b * 128 + g * 16:b * 128 + (g + 1) * 16] = 1.0
    return {
        "rowsel": rowsel.astype(ml_dtypes.bfloat16),
        "ident": ident,
        "identb": ident.astype(ml_dtypes.bfloat16),
        "mblk": mblk.astype(ml_dtypes.bfloat16),
        "B2": B2.astype(ml_dtypes.bfloat16),
        "Bsel": Bsel.astype(ml_dtypes.bfloat16),
        "ones32": np.ones((32, N), ml_dtypes.bfloat16),
        "onesf": np.ones((32, 128), np.float32),
    }


def _prep_inputs(h, W, att_a):
    """Host-side marshalling: shard h over cores, transpose to [b,d,n],
    fold attention vectors into P = [W_g @ a_j_g | W_g @ a_i_g]."""
    import ml_dtypes
    h = np.asarray(h, dtype=np.float32)
    W = np.asarray(W, dtype=np.float32)
    att_a = np.asarray(att_a, dtype=np.float32)
    P = np.empty((D, 24), dtype=np.float32)
    for g in range(N_HEADS):
        Wg = W[:, g * HID:(g + 1) * HID]
        P[:, g] = Wg @ att_a[g, HID:]      # a_j -> ej (rows 0:8 of EIJT)
        P[:, 8 + g] = Wg @ att_a[g, :HID]  # a_i -> ei
        P[:, 16 + g] = P[:, g] + P[:, 8 + g]  # ei+ej diag scores
    Wb = W.astype(ml_dtypes.bfloat16)
    cst = _consts()
    in_maps = []
    for core in range(CORES):
        hs = h[core * BPC:(core + 1) * BPC]                # [4, 512, 128]
        hTs = np.ascontiguousarray(hs.transpose(0, 2, 1))  # [4, 128, 512]
        hnat = np.ascontiguousarray(
            hs.reshape(BPC, NCH, 128, D).transpose(0, 2, 1, 3)
        ).astype(ml_dtypes.bfloat16)                       # [4, 128, 4, 128]
        m = {"hT": hTs, "hTb": hTs.astype(ml_dtypes.bfloat16),
             "hnat": hnat, "W": Wb, "P": P}
        m.update(cst)
        in_maps.append(m)
    return in_maps


def kernel(h, W, att_a):
    nc = _get_graph()
    in_maps = _prep_inputs(h, W, att_a)
    res = run_bass_kernel_spmd(nc, in_maps, list(range(CORES))).results
    outs = [r["out"].transpose(0, 2, 1, 3) for r in res]  # [4,H,n,d] each
    return np.ascontiguousarray(np.concatenate(outs, axis=0))


# revision 35
# speedup vs baseline: 1.1312x; 1.1312x over previous
"""MultiHeadGAT Trainium2 kernel (8 NeuronCores, data-parallel over batch).

Reference computation (per batch b of 32, n=512 nodes, d=128 feats, H=8 heads,
HID=64, top-k=16, leaky=0.2):
    h' = (h @ W).reshape(n, H, HID)                      # projection
    ei[g,i] = h'[i,g,:] . a_i[g];  ej[g,j] = h'[j,g,:] . a_j[g]
    e[g,i,j] = leaky_relu(ei[g,i] + ej[g,j])
    mask = topk_16(e, axis=j) | eye(n)
    attn = softmax(where(mask, e, -1e9))
    out = elu(attn @ h')

Structure exploited (rank-17 trick): the top-16 column set J_g is
row-independent (leaky_relu is monotone), so attention reduces to 16 shared
candidates per head plus the diagonal.

Candidate scores live in a TRANSPOSED layout [(head,cand)=128 part x n=512]:
  * cand = t_c + ei_n comes out of ONE K=64 PE matmul (stacked selector)
  * exp(leaky(cand)) = max(exp(s), exp(0.2 s)) - two ACTs + one bf16 max
  * the result IS the stationary operand of the output matmul (no transposes)
Other key moves:
  * V gather via associativity: V = (h^T S)^T @ W  (gather in d=128 space)
  * diagonal term dt = pd * h' reads the projection straight out of PSUM
    (projection runs late; no separate PSUM->SBUF evict pass)
  * softmax reciprocal via DVE reciprocal_approx_fast (no ACT table switch)
  * den for all 4 batches accumulated into one [32,N] PSUM group via
    per-batch block-selector lhsT constants
  * ELU via elu(y) = min(exp(y)-1, relu(y)); per-batch variant split
    balances Scalar vs Vector load
  * program order keeps PE ahead of Scalar/DVE consumers (HAM warm)
"""
import sys
import numpy as np

sys.path.insert(0, "/opt/trn_rl_repo")

from contextlib import ExitStack

import concourse.bass as bass
import concourse.tile as tile
from concourse import bacc, mybir
from concourse.bass_utils import run_bass_kernel_spmd

f32 = mybir.dt.float32
bf16 = mybir.dt.bfloat16
AX = mybir.AxisListType
ALU = mybir.AluOpType
AF = mybir.ActivationFunctionType

N_HEADS = 8
HID = 64
TOP_K = 16
SLOPE = 0.2
BS, N, D = 32, 512, 128
CORES = 8
BPC = BS // CORES          # batches per core = 4
NCH = N // 128             # n-chunks = 4
GD = N_HEADS * HID         # 512


def _bc(ap, insert_at, counts_steps):
    """Insert [step, count] dims into an AP at position insert_at."""
    new = list(ap.ap)
    for step, count in reversed(counts_steps):
        new.insert(insert_at, [step, count])
    return bass.AP(ap.tensor, ap.offset, new)


def build_graph():
    nc = bacc.Bacc("TRN2", target_bir_lowering=False, debug=False)

    hT_ext = nc.dram_tensor("hT", [BPC, D, N], f32, kind="ExternalInput")
    hTb_ext = nc.dram_tensor("hTb", [BPC, D, N], bf16, kind="ExternalInput")
    hnat_ext = nc.dram_tensor("hnat", [BPC, 128, NCH, D], bf16,
                              kind="ExternalInput")
    W_ext = nc.dram_tensor("W", [D, GD], bf16, kind="ExternalInput")
    P_ext = nc.dram_tensor("P", [D, 24], f32, kind="ExternalInput")
    # host-precomputed constants
    rowsel_ext = nc.dram_tensor("rowsel", [32, BPC * 128], bf16,
                                kind="ExternalInput")
    ident_ext = nc.dram_tensor("ident", [128, 128], f32, kind="ExternalInput")
    identb_ext = nc.dram_tensor("identb", [128, 128], bf16,
                                kind="ExternalInput")
    mblk_ext = nc.dram_tensor("mblk", [128, GD], bf16, kind="ExternalInput")
    b2_ext = nc.dram_tensor("B2", [128, BPC, 32], bf16, kind="ExternalInput")
    bsel_ext = nc.dram_tensor("Bsel", [32, BPC * 128], bf16,
                              kind="ExternalInput")
    ones_ext = nc.dram_tensor("ones32", [32, N], bf16, kind="ExternalInput")
    onesf_ext = nc.dram_tensor("onesf", [32, 128], f32, kind="ExternalInput")
    out_ext = nc.dram_tensor("out", [BPC, N, N_HEADS, HID], bf16,
                             kind="ExternalOutput")
    hT = hT_ext.ap()
    hTb = hTb_ext.ap()
    hnat = hnat_ext.ap()
    outap = out_ext.ap()

    with tile.TileContext(nc) as tc, ExitStack() as ctx:
        const = ctx.enter_context(tc.tile_pool(name="const", bufs=1))
        sb = ctx.enter_context(tc.tile_pool(name="sb", bufs=2))
        sb3 = ctx.enter_context(tc.tile_pool(name="sb3", bufs=3))
        sb4 = ctx.enter_context(tc.tile_pool(name="sb4", bufs=BPC))
        # PSUM (8 banks): o 2x2 + mm 2x1 + sm 1 + den 1
        ps = ctx.enter_context(tc.tile_pool(name="ps", bufs=2, space="PSUM"))

        # ---------------- constants (host-precomputed, DMA'd) ----------
        P_sb = const.tile([128, 24], f32)
        nc.sync.dma_start(P_sb[:], P_ext.ap())
        W_sb = const.tile([128, GD], bf16)
        nc.scalar.dma_start(W_sb[:], W_ext.ap())
        ident = const.tile([128, 128], f32)
        nc.scalar.dma_start(ident[:], ident_ext.ap())
        identb = const.tile([128, 128], bf16)
        nc.scalar.dma_start(identb[:], identb_ext.ap())
        mblk = const.tile([128, GD], bf16)
        nc.scalar.dma_start(mblk[:], mblk_ext.ap())
        B2 = const.tile([128, BPC, 32], bf16)
        nc.scalar.dma_start(B2[:], b2_ext.ap())
        Bsel = const.tile([32, BPC * 128], bf16)
        nc.scalar.dma_start(Bsel[:], bsel_ext.ap())
        # s-matmul operands in hi+lo bf16 (f32 PE matmuls run as 2-pass
        # LOW_HIGH at ~8x the bf16 cost); all constants, single DMAs
        ones32 = const.tile([32, N], bf16)
        nc.scalar.dma_start(ones32[:], ones_ext.ap())
        rowsel = const.tile([32, BPC * 128], bf16)
        nc.scalar.dma_start(rowsel[:], rowsel_ext.ap())
        onesf = const.tile([32, 128], f32)
        nc.scalar.dma_start(onesf[:], onesf_ext.ap())

        # ---------------- stage A: scores ------------------------------
        T = const.tile([32, N], f32)       # ej rows: (b,g) x n
        sd = const.tile([32, N], f32)      # ei+ej rows (diag scores)
        EI = const.tile([32, N], f32)      # ei rows
        hT_sb = []
        hTb_sb = []
        hnat_sb = []
        sc_sb = []
        for b in range(BPC):
            ht = sb4.tile([128, N], f32, tag="ht")
            nc.sync.dma_start(ht[:, 0:N // 2], hT[b][:, 0:N // 2])
            nc.sync.dma_start(ht[:, N // 2:], hT[b][:, N // 2:])
            htb = sb4.tile([128, N], bf16, tag="htb")
            nc.scalar.dma_start(htb[:], hTb[b])
            hnt = sb4.tile([128, NCH, D], bf16, tag="hnat")
            nc.gpsimd.dma_start(hnt[:], hnat[b])
            hT_sb.append(ht)
            hTb_sb.append(htb)
            hnat_sb.append(hnt)

            # scores EIJT[(ij,g), n] = P^T @ hT   (f32; bf16 scores fail:
            # 0.16 rel err from top-k set instability)
            sc_ps = ps.tile([24, N], f32, tag="mm")
            nc.tensor.matmul(sc_ps[:], P_sb[:], ht[:], start=True, stop=True)
            sc = sb4.tile([24, N], f32, tag="sc")
            nc.vector.tensor_copy(sc[:], sc_ps[:])
            sc_sb.append(sc)
            nc.gpsimd.dma_start(T[b * 8:(b + 1) * 8, :], sc[0:8, :])
            nc.gpsimd.dma_start(EI[b * 8:(b + 1) * 8, :], sc[8:16, :])
            nc.gpsimd.dma_start(sd[b * 8:(b + 1) * 8, :], sc[16:24, :])

        # ---------------- stage B: top-16 + diagonal terms -------------
        vals = const.tile([32, 16], f32)
        T2 = const.tile([32, N], f32)
        nc.vector.max(vals[:, 0:8], T[:])
        nc.vector.match_replace(T2[:], vals[:, 0:8], T[:], -1e30)
        nc.vector.max(vals[:, 8:16], T2[:])

        # diag: pdiag = exp(leaky(ei+ej)) * (ej < t16)   [32, 512] folded
        sdl = const.tile([32, N], f32)
        nc.vector.scalar_tensor_tensor(sdl[:], sd[:], SLOPE, sd[:],
                                       op0=ALU.mult, op1=ALU.max)
        expd = const.tile([32, N], f32)
        nc.scalar.activation(expd[:], sdl[:], AF.Exp)
        pdiag = const.tile([32, N], f32)
        nc.vector.scalar_tensor_tensor(pdiag[:], T[:], vals[:, 15:16],
                                       expd[:], op0=ALU.is_lt, op1=ALU.mult)

        # hi/lo splits for the bf16 s-matmul and rbc-matmul
        EIh = const.tile([32, N], bf16)
        nc.vector.tensor_copy(EIh[:], EI[:])
        EIl = const.tile([32, N], bf16)
        nc.vector.tensor_tensor(EIl[:], EI[:], EIh[:], op=ALU.subtract)
        vals_h = const.tile([32, 16], bf16)
        nc.vector.tensor_copy(vals_h[:], vals[:])
        vals_l = const.tile([32, 16], bf16)
        nc.vector.tensor_tensor(vals_l[:], vals[:], vals_h[:],
                                op=ALU.subtract)

        # ej in node layout for the one-hot gather: 4 transposes of T
        ejt_ps = ps.tile([128, NCH, 32], f32, tag="sm", bufs=1)
        for c in range(NCH):
            nc.tensor.transpose(ejt_ps[:, c, :],
                                T[:, c * 128:(c + 1) * 128],
                                ident[0:32, 0:32])
        ejn = const.tile([128, NCH, 32], f32)
        nc.vector.tensor_copy(ejn[:], ejt_ps[:])

        # ------------- stage C1a per batch: scores -> pex, S -----------
        pex_sb = []
        S_sb = []
        for b in range(BPC):
            # rhsb_*[k,(g,c)] = vals_*[k,c] * (k == b*8+g)
            id_sl = ident[0:32, b * 8:(b + 1) * 8].broadcast_to([32, 8, 16])
            rbh = sb.tile([32, N_HEADS, 16], bf16, tag="rbh")
            nc.vector.tensor_tensor(
                rbh[:], _bc(vals_h[:, 0:16], 1, [[0, N_HEADS]]), id_sl,
                op=ALU.mult)
            rbl = sb.tile([32, N_HEADS, 16], bf16, tag="rbl")
            nc.vector.tensor_tensor(
                rbl[:], _bc(vals_l[:, 0:16], 1, [[0, N_HEADS]]), id_sl,
                op=ALU.mult)

            # s[(g,c), n] = ei[b,g,n] + t[(g,c)]: 4 accumulating bf16 mms
            s_ps = ps.tile([128, N], f32, tag="mm")
            nc.tensor.matmul(s_ps[:], rbh[:].rearrange("k g c -> k (g c)"),
                             ones32[:], start=True, stop=False)
            nc.tensor.matmul(s_ps[:], rbl[:].rearrange("k g c -> k (g c)"),
                             ones32[:], start=False, stop=False)
            nc.tensor.matmul(s_ps[:], rowsel[:, b * 128:(b + 1) * 128],
                             EIh[:], start=False, stop=False)
            nc.tensor.matmul(s_ps[:], rowsel[:, b * 128:(b + 1) * 128],
                             EIl[:], start=False, stop=True)

            # one-hot prep: vbc[p,(g,c)] = vals[b*8+g, c] broadcast
            # (must be bit-exact vals for the f32 is_equal match -> f32 mm)
            rhsbf = sb.tile([32, N_HEADS, 16], f32, tag="rhsbf")
            nc.vector.tensor_tensor(
                rhsbf[:], _bc(vals[:, 0:16], 1, [[0, N_HEADS]]), id_sl,
                op=ALU.mult)
            vbc_ps = ps.tile([128, 128], f32, tag="sm", bufs=1)
            nc.tensor.matmul(vbc_ps[:], onesf[:],
                             rhsbf[:].rearrange("k g c -> k (g c)"),
                             start=True, stop=True)
            vbc = sb.tile([128, 128], f32, tag="vbc")
            nc.vector.tensor_copy(vbc[:], vbc_ps[:])

            # exp(leaky(s)) = max(exp(s), exp(0.2 s))
            e1 = sb.tile([128, N], bf16, tag="e1")
            nc.scalar.activation(e1[:], s_ps[:], AF.Exp)
            e2 = sb.tile([128, N], bf16, tag="e2")
            nc.scalar.activation(e2[:], s_ps[:], AF.Exp, scale=SLOPE)
            pex = sb4.tile([128, N], bf16, tag="pex")
            nc.vector.tensor_tensor(pex[:], e1[:], e2[:], op=ALU.max)
            pex_sb.append(pex)

            # one-hot S[n, (g,c)] = (ej[n,g] == vals[b*8+g, c])
            S = sb4.tile([128, NCH, 128], bf16, tag="S")
            ej_bc = _bc(ejn[:, :, b * 8:(b + 1) * 8], 3, [[0, 16]])
            vbc_bc = _bc(vbc[:].rearrange("p (g c) -> p g c", g=N_HEADS),
                         1, [[0, NCH]])
            nc.vector.tensor_tensor(
                S[:].rearrange("p c (g x) -> p c g x", g=N_HEADS),
                ej_bc, vbc_bc, op=ALU.is_equal)
            S_sb.append(S)

        # ------------- stage C1b per batch: V gather + den -------------
        den_ps = ps.tile([32, N], f32, tag="den", bufs=1)
        hblk_sb = []
        for b in range(BPC):
            # V = (h^T S)^T W ; gather in d-space then project
            hS_ps = ps.tile([128, 128], f32, tag="sm", bufs=1)
            for c in range(NCH):
                nc.tensor.matmul(hS_ps[:], hnat_sb[b][:, c, :],
                                 S_sb[b][:, c, :],
                                 start=(c == 0), stop=(c == NCH - 1))
            hS = sb.tile([128, 128], bf16, tag="hS")
            nc.vector.tensor_copy(hS[:], hS_ps[:])
            V_ps = ps.tile([128, GD], f32, tag="mm")
            nc.tensor.matmul(V_ps[:], hS[:], W_sb[:], start=True, stop=True)
            hblk = sb4.tile([128, GD], bf16, tag="hblk")
            nc.vector.tensor_tensor(hblk[:], V_ps[:], mblk[:], op=ALU.mult)
            hblk_sb.append(hblk)

            # den[(b,g), n] = sum_c pex, accumulated into one [32,N] group
            nc.tensor.matmul(den_ps[:], B2[:, b, :], pex_sb[b][:],
                             start=(b == 0), stop=(b == BPC - 1),
                             skip_group_check=True)

        # ---------------- stage R: reciprocal + pd (global) ------------
        denf = const.tile([32, N], f32)
        nc.vector.tensor_tensor(denf[:], den_ps[:], pdiag[:], op=ALU.add)
        recip = const.tile([32, N], f32)
        nc.vector.reciprocal_approx_fast(recip[:], denf[:])
        r_hi = const.tile([32, N], bf16)
        nc.vector.tensor_copy(r_hi[:], recip[:])
        r_lo = const.tile([32, N], bf16)
        nc.vector.tensor_tensor(r_lo[:], recip[:], r_hi[:], op=ALU.subtract)
        pd_t = const.tile([32, N], bf16)
        nc.gpsimd.tensor_tensor(pd_t[:], pdiag[:], recip[:], op=ALU.mult)

        # pdn[n, (c, b, g)]: 4 PE transposes of pd_t
        pdn_ps = ps.tile([128, NCH, 32], bf16, tag="sm", bufs=1)
        for c in range(NCH):
            nc.tensor.transpose(pdn_ps[:, c, :],
                                pd_t[:, c * 128:(c + 1) * 128],
                                identb[0:32, 0:32])
        pdn = const.tile([128, NCH, 32], bf16)
        nc.vector.tensor_copy(pdn[:], pdn_ps[:])

        # ---------------- stage C2 per batch: qn, proj, out, elu -------
        qn_sb = []
        for b in range(BPC):
            # recip broadcast to (g,c) partitions; qn = pex * recip
            rbc_ps = ps.tile([128, N], f32, tag="mm")
            nc.tensor.matmul(rbc_ps[:], Bsel[:, b * 128:(b + 1) * 128],
                             r_hi[:], start=True, stop=False)
            nc.tensor.matmul(rbc_ps[:], Bsel[:, b * 128:(b + 1) * 128],
                             r_lo[:], start=False, stop=True)
            qn = sb4.tile([128, N], bf16, tag="qn")
            nc.vector.tensor_tensor(qn[:], pex_sb[b][:], rbc_ps[:],
                                    op=ALU.mult)
            qn_sb.append(qn)

        for b in range(BPC):
            qn = qn_sb[b]
            # (re)projection per half; dt = pd * h' straight from PSUM
            dt = sb.tile([128, NCH, GD], bf16, tag="dt")
            for h in range(2):
                hp_ps = ps.tile([128, 2, GD], f32, tag="o")
                for k in range(2):
                    c = 2 * h + k
                    nc.tensor.matmul(hp_ps[:, k, :],
                                     hTb_sb[b][:, c * 128:(c + 1) * 128],
                                     W_sb[:], start=True, stop=True)
                pdn_b = _bc(
                    _bc(pdn[:, 2 * h:2 * h + 2, b * 8:(b + 1) * 8],
                        3, [[0, HID]]), 0, [])
                nc.vector.tensor_tensor(
                    dt[:, 2 * h:2 * h + 2, :].rearrange(
                        "p c (g x) -> p c g x", g=N_HEADS),
                    hp_ps[:].rearrange("p c (g x) -> p c g x", g=N_HEADS),
                    pdn_b, op=ALU.mult)

            # output halves: attn matmul + diag add, then ELU
            for h in range(2):
                o_ps = ps.tile([128, 2, GD], f32, tag="o")
                for k in range(2):
                    c = 2 * h + k
                    nc.tensor.matmul(o_ps[:, k, :],
                                     qn[:, c * 128:(c + 1) * 128],
                                     hblk_sb[b][:], start=True, stop=False)
                for k in range(2):
                    c = 2 * h + k
                    nc.tensor.matmul(o_ps[:, k, :], identb[:], dt[:, c, :],
                                     start=False, stop=True)
                vex = sb3.tile([128, 2, GD], bf16, tag="vex")
                nc.scalar.activation(vex[:], o_ps[:], AF.Exp)
                ot = sb3.tile([128, 2, GD], bf16, tag="ot")
                if b < 2:
                    # scalar-heavy: ry on ACT, all-bf16 STT
                    ry = sb3.tile([128, 2, GD], bf16, tag="ry")
                    nc.scalar.activation(ry[:], o_ps[:], AF.Relu)
                    nc.vector.scalar_tensor_tensor(ot[:], vex[:], 1.0, ry[:],
                                                   op0=ALU.subtract,
                                                   op1=ALU.min)
                else:
                    # vector-heavy: vexm1 via 4x TS, mixed STT vs PSUM
                    vm1 = sb3.tile([128, 2, GD], bf16, tag="vm1")
                    nc.vector.tensor_scalar(vm1[:], vex[:], 1.0, None,
                                            op0=ALU.subtract)
                    nc.vector.scalar_tensor_tensor(ot[:], o_ps[:], 0.0,
                                                   vm1[:], op0=ALU.max,
                                                   op1=ALU.min)
                nc.sync.dma_start(
                    outap[b, 2 * h * 128:(2 * h + 2) * 128].rearrange(
                        "(k p) g x -> p k g x", k=2),
                    ot[:].rearrange("p k (g x) -> p k g x", g=N_HEADS))

    nc.compile()
    return nc


_CACHE = {}


def _get_graph():
    if "nc" not in _CACHE:
        _CACHE["nc"] = build_graph()
    return _CACHE["nc"]


def _consts():
    import ml_dtypes
    rowsel = np.zeros((32, BPC * 128), np.float32)
    for b in range(BPC):
        for g in range(N_HEADS):
            rowsel[b * 8 + g, b * 128 + g * 16:b * 128 + (g + 1) * 16] = 1.0
    ident = np.eye(128, dtype=np.float32)
    mblk = np.zeros((128, GD), np.float32)
    for g in range(N_HEADS):
        mblk[g * 16:(g + 1) * 16, g * HID:(g + 1) * HID] = 1.0
    B2 = np.zeros((128, BPC, 32), np.float32)
    for b in range(BPC):
        for g in range(N_HEADS):
            B2[g * 16:(g + 1) * 16, b, b * 8 + g] = 1.0
    Bsel = np.zeros((32, BPC * 128), np.float32)
    for b in range(BPC):
        for g in range(N_HEADS):
            Bsel[b * 8 + g, b * 128 + g * 16:b * 128 + (g + 1) * 16] = 1.0
    return {
        "rowsel": rowsel.astype(ml_dtypes.bfloat16),
        "ident": ident,
        "identb": ident.astype(ml_dtypes.bfloat16),
        "mblk": mblk.astype(ml_dtypes.bfloat16),
        "B2": B2.astype(ml_dtypes.bfloat16),
        "Bsel": Bsel.astype(ml_dtypes.bfloat16),
        "ones32": np.ones((32, N), ml_dtypes.bfloat16),
        "onesf": np.ones((32, 128), np.float32),
    }


def _prep_inputs(h, W, att_a):
    """Host-side marshalling: shard h over cores, transpose to [b,d,n],
    fold attention vectors into P = [W_g @ a_j_g | W_g @ a_i_g]."""
    import ml_dtypes
    h = np.asarray(h, dtype=np.float32)
    W = np.asarray(W, dtype=np.float32)
    att_a = np.asarray(att_a, dtype=np.float32)
    P = np.empty((D, 24), dtype=np.float32)
    for g in range(N_HEADS):
        Wg = W[:, g * HID:(g + 1) * HID]
        P[:, g] = Wg @ att_a[g, HID:]      # a_j -> ej (rows 0:8 of EIJT)
        P[:, 8 + g] = Wg @ att_a[g, :HID]  # a_i -> ei
        P[:, 16 + g] = P[:, g] + P[:, 8 + g]  # ei+ej diag scores
    Wb = W.astype(ml_dtypes.bfloat16)
    cst = _consts()
    in_maps = []
    for core in range(CORES):
        hs = h[core * BPC:(core + 1) * BPC]                # [4, 512, 128]
        hTs = np.ascontiguousarray(hs.transpose(0, 2, 1))  # [4, 128, 512]
        hnat = np.ascontiguousarray(
            hs.reshape(BPC, NCH, 128, D).transpose(0, 2, 1, 3)
        ).astype(ml_dtypes.bfloat16)                       # [4, 128, 4, 128]
        m = {"hT": hTs, "hTb": hTs.astype(ml_dtypes.bfloat16),
             "hnat": hnat, "W": Wb, "P": P}
        m.update(cst)
        in_maps.append(m)
    return in_maps


def kernel(h, W, att_a):
    nc = _get_graph()
    in_maps = _prep_inputs(h, W, att_a)
    res = run_bass_kernel_spmd(nc, in_maps, list(range(CORES))).results
    outs = [r["out"].transpose(0, 2, 1, 3) for r in res]  # [4,H,n,d] each
    return np.ascontiguousarray(np.concatenate(outs, axis=0))
